# revision 1
# baseline (speedup 1.0000x reference)
"""Trainium2 Bass kernel for nn_DeepWarping (8-core data parallel).

Math notes (exploited structure, verified against the reference):
  - logprior_rotate_matrix M is circulant: M[i,j] = f((j-i) % 36), f = M[0,:].
  - template_log groups (i,j) pairs by k = (j-i) % 36, so the double
    logsumexp over the [36,36] grid collapses to a 36-point circular
    correlation: W[k] = sum_i exp(ll1[i]) * exp(ll2[(i+k)%36]), and
    post_rot[k] = W[k]*exp(f(k)) / sum_k' W[k']*exp(f(k')).
  - warped = T[idx[b]] @ inp[b,s] with idx = 30 + round(yaw*180/pi).  The
    whole transform bank is DMA'd once in [j, (a,i)] layout and each batch's
    matrix is selected with a register-offset dynamic slice as the matmul's
    moving operand (PE), so no gather / relayout is needed.

Hardware pitfalls baked in (all verified on HW):
  - a step-0 (broadcast) free dim on a DVE operand must be INNERMOST;
  - a single matmul's PSUM output must not cross a 2KB bank boundary;
  - DVE f32->int32 tensor_copy rounds to nearest (matches jnp.round);
  - tensor_tensor_reduce is broken on HW (unrecoverable exec error);
  - walrus rejects register offsets on the stationary (lhsT) operand, the
    moving operand accepts them.

Sharding: pure data parallel over the batch dim, 16 batches per core; each
core works on 112 = 16*7 (b,s) rows mapped to SBUF partitions.
"""

import numpy as np

import concourse.bacc as bacc
import concourse.bass as bass
import concourse.mybir as mybir
import concourse.tile as tile
from concourse.bass_utils import run_bass_kernel_spmd

NB = 36          # angle bins
NA = 61          # transform bank size
B, S = 128, 7    # full batch / seq
NCORES = 8
BPC = B // NCORES          # batches per core (16)
P = BPC * S                # (b,s) rows per core (112)
EXT = 2 * NB - 1           # 71
OC = 2 * NB + 2            # 74 output cols
DEG = 57.29577951308232    # 180/pi

# bundle column layout (rows 0:P): ll1 | ll2e | expf | pop2i | eps2
C_LL1, C_LL2E, C_EXPF, C_POP, C_EPS = 0, NB, NB + EXT, NB + EXT + NB, NB + EXT + NB + 2 * NB
BUND = C_EPS + 2           # 217
# bank columns (rows 0:NB): transform bank [j,(a,i)] | inpT
BANKW = NA * NB + P        # 2308

_DT = mybir.dt.float32


def _fv(base, dims):
    """View of an SBUF tile with custom free-dim (step,count) pairs."""
    return bass.AP(
        tensor=base.tensor,
        offset=base.offset,
        ap=[list(base.ap[0])] + [list(d) for d in dims],
    )


def _emit(nc, n_iters=1):
    dt = _DT
    d_yaw1 = nc.dram_tensor("yaw1", [1, BPC], dt, kind="ExternalInput")
    d_bank = nc.dram_tensor("bank", [NB, BANKW], dt, kind="ExternalInput")
    d_bund = nc.dram_tensor("bund", [P, BUND], dt, kind="ExternalInput")
    d_out = nc.dram_tensor("out", [P, OC], dt, kind="ExternalOutput")

    alu = mybir.AluOpType
    act = mybir.ActivationFunctionType
    X = mybir.AxisListType.X

    with tile.TileContext(nc) as tc:
        with (
            tc.tile_pool(name="sb", bufs=1) as sb,
            tc.tile_pool(name="ps", bufs=1, space="PSUM") as ps,
        ):
            for _it in range(n_iters):
                yaw1 = sb.tile([1, BPC], dt, tag="yaw1")
                bank = sb.tile([NB, BANKW], dt, tag="bank")
                bund = sb.tile([P, BUND], dt, tag="bund")
                d = sb.tile([1, BPC], dt, tag="d")
                di = sb.tile([1, BPC], mybir.dt.int32, tag="di")
                df = sb.tile([1, BPC], dt, tag="df")
                delta = sb.tile([1, BPC], dt, tag="delta")
                cp = sb.tile([1, BPC], dt, tag="cp")
                dfix = sb.tile([1, BPC], dt, tag="dfix")
                d36 = sb.tile([1, BPC], dt, tag="d36")
                i36 = sb.tile([1, BPC], mybir.dt.int32, tag="i36")
                t1 = sb.tile([P, NB], dt, tag="t1")
                t2e = sb.tile([P, EXT], dt, tag="t2e")
                prd = sb.tile([P, NB * NB], dt, tag="prd")
                w = sb.tile([P, NB], dt, tag="w")
                wf = sb.tile([P, NB], dt, tag="wf")
                sz = sb.tile([P, 1], dt, tag="sz")
                rz = sb.tile([P, 1], dt, tag="rz")
                post = sb.tile([P, NB], dt, tag="post")
                prdv = sb.tile([P, 2 * NB], dt, tag="prdv")
                vecu = sb.tile([P, 2], dt, tag="vecu")
                vec = sb.tile([P, 2], dt, tag="vec")
                sqv = sb.tile([P, 2], dt, tag="sqv")
                n2 = sb.tile([P, 1], dt, tag="n2")
                lnn = sb.tile([P, 1], dt, tag="lnn")
                rn = sb.tile([P, 1], dt, tag="rn")
                vclip = sb.tile([P, 2], dt, tag="vclip")
                outb = sb.tile([P, 2 + NB], dt, tag="outb")
                wsb = sb.tile([S, BPC * NB], dt, tag="wsb")
                wpsA = ps.tile([S, BPC * NB // 2], dt, tag="wpsA")
                wpsB = ps.tile([S, BPC * NB // 2], dt, tag="wpsB")

                # ---- loads ----
                nc.sync.dma_start(yaw1[:], d_yaw1[:])
                nc.sync.dma_start(bank[:], d_bank[:])
                nc.sync.dma_start(bund[:], d_bund[:])
                ll1 = bund[:, C_LL1:C_LL1 + NB]
                ll2e = bund[:, C_LL2E:C_LL2E + EXT]
                expf = bund[:, C_EXPF:C_EXPF + NB]
                eps2 = bund[:, C_EPS:C_EPS + 2]

                # ---- yaw -> per-batch bank column offset (36*idx) ----
                # f32->i32 convert rounds-to-nearest on HW but truncates in
                # CoreSim; the is_ge fix-up yields round() under both modes
                # (d is always > 0 here).
                nc.vector.tensor_scalar(d[:], yaw1[:], DEG, 30.0, alu.mult, alu.add)
                nc.vector.tensor_copy(di[:], d[:])
                nc.vector.tensor_copy(df[:], di[:])
                nc.vector.tensor_sub(delta[:], d[:], df[:])
                nc.vector.tensor_scalar(cp[:], delta[:], 0.5, None, alu.is_ge)
                nc.vector.tensor_add(dfix[:], df[:], cp[:])
                nc.vector.tensor_scalar(d36[:], dfix[:], float(NB), None, alu.mult)
                nc.vector.tensor_copy(i36[:], d36[:])

                # ---- warped^T[s, (b,i)] via dynamic-slice matmuls on PE ----
                half = BPC // 2
                for b in range(BPC):
                    tgt = wpsA if b < half else wpsB
                    bb = b if b < half else b - half
                    with nc.tensor.register(f"off{_it}_{b}") as r:
                        nc.tensor.reg_load(r, i36[0:1, b:b + 1])
                        off = nc.tensor.snap(r, min_val=0, max_val=(NA - 1) * NB)
                        nc.tensor.matmul(
                            tgt[:, NB * bb:NB * (bb + 1)],
                            bank[:, NA * NB + S * b:NA * NB + S * (b + 1)],
                            bank[:, bass.ds(off, NB)],
                            start=True, stop=True,
                        )
                nc.scalar.copy(wsb[:, :NB * half], wpsA[:])
                nc.scalar.copy(wsb[:, NB * half:], wpsB[:])

                # ---- circular correlation of exp(ll1), exp(ll2) ----
                nc.scalar.activation(t1[:], ll1, act.Exp)
                nc.scalar.activation(t2e[:], ll2e, act.Exp)
                # PRD[p, i*NB+k] = t1[p,i] * t2e[p,i+k]; step-0 dim innermost
                prd3 = prd[:].rearrange("p (i k) -> p i k", i=NB)
                nc.vector.tensor_mul(prd3, _fv(t1[:], [[1, NB], [0, NB]]),
                                     _fv(t2e[:], [[1, NB], [1, NB]]))
                # W[p,k] = sum_i PRD[p,i,k] via strided-inner view [p, k, i]
                nc.vector.reduce_sum(w[:], _fv(prd[:], [[1, NB], [NB, NB]]), axis=X)

                # ---- posterior over rotations ----
                nc.vector.tensor_mul(wf[:], w[:], expf)
                nc.vector.reduce_sum(sz[:], wf[:], axis=X)
                nc.vector.reciprocal(rz[:], sz[:])
                nc.vector.tensor_scalar(post[:], wf[:], rz[:, :1], None, alu.mult)
                nc.scalar.activation(outb[:, 2:], post[:], act.Ln)

                # ---- population vector readout (pop2i is (k,c)-interleaved) ----
                pop2i = _fv(bund[:, C_POP:C_POP + 2 * NB], [[2, NB], [1, 2]])
                nc.vector.tensor_mul(prdv[:].rearrange("p (k c) -> p k c", k=NB),
                                     _fv(post[:], [[1, NB], [0, 2]]), pop2i)
                nc.vector.reduce_sum(vecu[:], _fv(prdv[:], [[1, 2], [2, NB]]), axis=X)
                nc.vector.tensor_add(vec[:], vecu[:], eps2)
                nc.vector.tensor_mul(sqv[:], vec[:], vec[:])
                nc.vector.reduce_sum(n2[:], sqv[:], axis=X)
                # 1/sqrt(n2) = Exp(-0.5*Ln(n2)) — keeps ACT on one table set
                nc.scalar.activation(lnn[:], n2[:], act.Ln)
                nc.scalar.activation(rn[:], lnn[:], act.Exp, scale=-0.5)
                nc.vector.tensor_scalar(vclip[:], vec[:], rn[:, :1], 1.0,
                                        alu.mult, alu.min)
                nc.vector.tensor_scalar(outb[:, 0:2], vclip[:], -1.0, None,
                                        alu.max)

                # ---- stores ----
                # warped: SBUF [s,(b,i)] -> DRAM out[(b,s), 0:NB]
                o_ap = d_out[:]
                dst_w = bass.AP(tensor=o_ap.tensor, offset=o_ap.offset,
                                ap=[[OC, S], [S * OC, BPC], [1, NB]])
                nc.sync.dma_start(dst_w, wsb[:].rearrange("s (b i) -> s b i", b=BPC))
                # vec+logpost: SBUF [p, 38] -> DRAM out[:, NB:]
                nc.sync.dma_start(d_out[:, NB:], outb[:])

    return nc


_NC_CACHE = {}


def _get_nc(n_iters=1):
    nc = _NC_CACHE.get(n_iters)
    if nc is None:
        nc = _emit(bacc.Bacc(None, target_bir_lowering=False), n_iters=n_iters)
        nc.compile()
        _NC_CACHE[n_iters] = nc
    return nc


def _in_maps(loglikelihood1, loglikelihood2, inp, yaw,
             transform_matrices, logprior_rotate_matrix, template_log,
             population_vector):
    f32 = np.float32
    ll1 = np.ascontiguousarray(loglikelihood1, f32)
    ll2 = np.ascontiguousarray(loglikelihood2, f32)
    inp = np.ascontiguousarray(inp, f32)
    yaw = np.ascontiguousarray(yaw, f32)
    T = np.ascontiguousarray(transform_matrices, f32)
    M = np.ascontiguousarray(logprior_rotate_matrix, f32)
    pop = np.ascontiguousarray(population_vector, f32)

    tbj2 = T.transpose(2, 0, 1).reshape(NB, NA * NB)     # [j, (a,i)]
    expf = np.tile(np.exp(M[0, :]).astype(f32), (P, 1))
    pop2i = np.tile(np.ascontiguousarray(pop.T).reshape(2 * NB), (P, 1))
    eps2 = np.tile(np.array([1e-8, 0.0], f32), (P, 1))

    maps = []
    for c in range(NCORES):
        bs = slice(BPC * c, BPC * (c + 1))
        l1 = ll1[bs].reshape(P, NB)
        l2 = ll2[bs].reshape(P, NB)
        bund = np.concatenate(
            [l1, l2, l2[:, :NB - 1], expf, pop2i, eps2], axis=1)
        bank = np.concatenate([tbj2, inp[bs].reshape(P, NB).T], axis=1)
        maps.append({
            "yaw1": yaw[bs].reshape(1, BPC),
            "bank": np.ascontiguousarray(bank),
            "bund": np.ascontiguousarray(bund),
        })
    return maps


def run(trace=False, **inputs):
    """Run on 8 NeuronCores; returns (full_output, exec_time_ns_or_None)."""
    nc = _get_nc()
    maps = _in_maps(**inputs)
    res = run_bass_kernel_spmd(nc, maps, list(range(NCORES)), trace=trace)
    parts = [res.results[c]["out"].reshape(BPC, S, OC) for c in range(NCORES)]
    out = np.concatenate(parts, axis=0).astype(np.float32)
    return out, res.exec_time_ns


def kernel(**inputs):
    return run(trace=False, **inputs)[0]



# revision 3
# speedup vs baseline: 1.5378x; 1.5378x over previous
"""Trainium2 Bass kernel for nn_DeepWarping (8-core data parallel).

Redesign vs the register-offset baseline (34.9us) — three structural moves:

1. Warp via SVD:  T[a] (61 yaw-indexed 36x36 matrices) compresses to
   rank R=12: T[a] ~= sum_h US[a,h] * C_h  (warped rel err ~1e-4; bf16
   inputs dominate at ~2e-3, vs the 2e-2 gate).  On device the warp is
   ONE static bf16 matmul Z2[p,(h,i)] = inp @ C_h plus a tiny DVE
   select with per-row weights uP[p,h] = poly_D(round(yaw*DEG)/26)
   (degree-14 fit of US at the 61 integer yaws).  No PE registers, no
   dynamic slices, no 316KB bank DMA.

2. Correlation via DFT on the PE:  W[p,k] = sum_i e1[p,i] e2[p,(i+k)%36]
   = (1/36) sum_n [A cos(2pi nk/36) - B sin(...)], A = R1R2+I1I2,
   B = I1R2-R1I2.  F-transforms and the inverse transform are small
   static matmuls; the prior row expf[k] and the normalizer column
   sz = sum_k wf[k] are folded into the iDFT stationary.  This replaces
   the [112,1296] DVE multiply + strided reduce (3.9us) with two
   [72,112] DVE products (0.6us).

3. Row-major tail via one PE transpose:  wfT+sz [37,112] transposes to
   [112,37]; then logpost = ln(wf) - ln(sz) is one ACT Ln + one DVE
   tensor_scalar (normalization is algebraically exact); the pop-vector
   readout runs on DVE views; 1/|v| = Exp(-0.5*Ln(n2)).

Act-table trick: Exp and Ln share table set 6 (natural_log_exp_and_
others) but the stock greedy pass picks per-function first-match sets
and pays 3 x 1.28us ACT_TABLE_LOADs.  _Bacc blanks every set that does
not contain both Exp and Ln (index-preserving) so exactly one load is
emitted.

Input DMAs are split across the two HWDGE queues (sync + scalar) so
they overlap.  Fixed NEFF prologue/epilogue (~9us of engine init +
semaphore teardown) dominates the remaining runtime.
"""

import numpy as np
import ml_dtypes

import concourse.bacc as bacc
import concourse.bass as bass
import concourse.mybir as mybir
import concourse.tile as tile
from concourse.bass_utils import run_bass_kernel_spmd

NB = 36          # angle bins
NA = 61          # yaw bank size
B, S = 128, 7    # full batch / seq
NCORES = 8
BPC = B // NCORES          # batches per core (16)
P = BPC * S                # (b,s) rows per core (112)
OC = 2 * NB + 2            # 74 output cols
DEG = 57.29577951308232    # 180/pi
R = 12                     # SVD rank of the transform bank
D = 15                     # poly degree+1 for the yaw->US fit

# A36 [36, 368]: X1T | X2T | dftA | dftB
C_X1T, C_X2T, C_DFTA, C_DFTB = 0, P, 2 * P, 2 * P + 72
W36 = 2 * P + 144
# A72 [72, 111]: idft1 | idft2 | ident37 (rows 0:37)
C_ID1, C_ID2, C_EYE = 0, 37, 74
W72 = 111
# A112 [112, 253]: yaw112 | coefRep (d-major) | pop2i
C_YAW, C_COEF, C_POP = 0, 1, 1 + D * R
W112 = 1 + D * R + 2 * NB
# B36 bf16 [36, 544]: Cbank [(h,i)] | inpT
C_CB, C_INP = 0, R * NB
WB36 = R * NB + P

_DT = mybir.dt.float32


class _Bacc(bacc.Bacc):
    """Bacc that restricts ACT table selection to sets containing both
    Exp and Ln, so the greedy per-activation chooser cannot alternate
    between an exp-only and an ln-only set (3 table loads -> 1)."""

    def insert_act_table_loads(self):
        import bass_rust as _bass_rust
        from concourse.hw_specs import get_activation_tables

        has_activation = any(
            isinstance(i, mybir.InstActivation)
            for b in self.main_func.blocks
            for i in b.instructions
        )
        if not has_activation:
            return
        need = {mybir.ActivationFunctionType.Exp, mybir.ActivationFunctionType.Ln}
        tables = [
            (name, funcs if need <= funcs else set())
            for name, funcs in get_activation_tables(self.m.arch).items()
        ]
        assert any(funcs for _, funcs in tables), "no table with Exp+Ln"
        _bass_rust.insert_act_table_loads(self, tables)


def _fv(base, dims):
    """View of an SBUF/PSUM tile with custom free-dim (step,count) pairs."""
    return bass.AP(
        tensor=base.tensor,
        offset=base.offset,
        ap=[list(base.ap[0])] + [list(d) for d in dims],
    )


def _emit(nc):
    dt = _DT
    bf = mybir.dt.bfloat16
    d_a36 = nc.dram_tensor("a36", [NB, W36], dt, kind="ExternalInput")
    d_a72 = nc.dram_tensor("a72", [72, W72], dt, kind="ExternalInput")
    d_a112 = nc.dram_tensor("a112", [P, W112], dt, kind="ExternalInput")
    d_b36 = nc.dram_tensor("b36", [NB, WB36], bf, kind="ExternalInput")
    d_out = nc.dram_tensor("out", [P, OC], dt, kind="ExternalOutput")

    alu = mybir.AluOpType
    act = mybir.ActivationFunctionType
    X = mybir.AxisListType.X

    with tile.TileContext(nc) as tc:
        with (
            tc.tile_pool(name="sb", bufs=1) as sb,
            tc.tile_pool(name="ps", bufs=1, space="PSUM") as ps,
        ):
            a36 = sb.tile([NB, W36], dt, tag="a36")
            a72 = sb.tile([72, W72], dt, tag="a72")
            a112 = sb.tile([P, W112], dt, tag="a112")
            b36 = sb.tile([NB, WB36], bf, tag="b36")
            t12e = sb.tile([NB, 2 * P], dt, tag="t12e")
            fsb = sb.tile([72, 336], dt, tag="fsb")
            dyw = sb.tile([P, 1], dt, tag="dyw")
            dyi = sb.tile([P, 1], mybir.dt.int32, tag="dyi")
            dyf = sb.tile([P, 1], dt, tag="dyf")
            powr = sb.tile([P, D], dt, tag="powr")
            q3 = sb.tile([P, D * R], dt, tag="q3")
            uP = sb.tile([P, R], dt, tag="uP")
            selq = sb.tile([P, R * NB], dt, tag="selq")
            p1 = sb.tile([72, P], dt, tag="p1")
            p2 = sb.tile([72, P], dt, tag="p2")
            asmT = sb.tile([37, P], dt, tag="asmT")
            lnrow = sb.tile([P, 37], dt, tag="lnrow")
            prdv = sb.tile([P, 2 * NB], dt, tag="prdv")
            vecu = sb.tile([P, 2], dt, tag="vecu")
            sqx = sb.tile([P, 1], dt, tag="sqx")
            n2 = sb.tile([P, 1], dt, tag="n2")
            lnn = sb.tile([P, 1], dt, tag="lnn")
            rn = sb.tile([P, 1], dt, tag="rn")
            vclip = sb.tile([P, 2], dt, tag="vclip")
            outb = sb.tile([P, OC], dt, tag="outb")
            z2 = ps.tile([P, R * NB], dt, tag="z2")
            fps = ps.tile([72, 336], dt, tag="fps")
            wfT = ps.tile([37, P], dt, tag="wfT")
            rowP = ps.tile([P, 37], dt, tag="rowP")

            # ---- loads: split across the two HWDGE queues ----
            nc.sync.dma_start(b36[:], d_b36[:])
            nc.sync.dma_start(a36[:], d_a36[:])
            nc.sync.dma_start(a72[:], d_a72[:])
            nc.scalar.dma_start(a112[:], d_a112[:])

            # ---- warp: Z2[p,(h,i)] = inp @ C_h (bf16 PE) ----
            nc.tensor.matmul(z2[:], b36[:, C_INP:C_INP + P],
                             b36[:, C_CB:C_CB + R * NB], start=True, stop=True)

            # ---- exp(ll1T | ll2T) on ACT, then DFT matmuls on PE ----
            nc.scalar.activation(t12e[:], a36[:, :2 * P], act.Exp)
            nc.tensor.matmul(fps[:, 0:224], a36[:, C_DFTA:C_DFTA + 72],
                             t12e[:], start=True, stop=True)
            nc.tensor.matmul(fps[:, 224:336], a36[:, C_DFTB:C_DFTB + 72],
                             t12e[:, P:2 * P], start=True, stop=True)

            # ---- yaw -> poly powers -> uP = powr @ coef (DVE) ----
            # t = (round(yaw*DEG + 30) - 30)/26; shift keeps the cast
            # argument positive (f32->i32 copy rounds-to-nearest on HW).
            nc.vector.tensor_scalar(dyw[:], a112[:, C_YAW:C_YAW + 1],
                                    DEG, 30.0, alu.mult, alu.add)
            nc.vector.tensor_copy(dyi[:], dyw[:])
            nc.vector.tensor_copy(dyf[:], dyi[:])
            nc.vector.tensor_scalar(powr[:, 1:2], dyf[:], 1.0 / 26.0,
                                    -30.0 / 26.0, alu.mult, alu.add)
            nc.vector.tensor_scalar(powr[:, 0:1], dyf[:], 0.0, 1.0,
                                    alu.mult, alu.add)
            nc.vector.tensor_mul(powr[:, 2:3], powr[:, 1:2], powr[:, 1:2])
            nc.vector.tensor_mul(powr[:, 3:5], powr[:, 1:3],
                                 _fv(powr[:, 2:3], [[0, 2]]))
            nc.vector.tensor_mul(powr[:, 5:9], powr[:, 1:5],
                                 _fv(powr[:, 4:5], [[0, 4]]))
            nc.vector.tensor_mul(powr[:, 9:D], powr[:, 1:7],
                                 _fv(powr[:, 8:9], [[0, D - 9]]))
            # q3[p,(d,h)] = powr[p,d] * coef[d,h]; reduce over d -> uP
            nc.vector.tensor_mul(q3[:].rearrange("p (d h) -> p d h", d=D),
                                 _fv(powr[:], [[1, D], [0, R]]),
                                 _fv(a112[:, C_COEF:C_COEF + D * R],
                                     [[R, D], [1, R]]))
            nc.vector.reduce_sum(uP[:], _fv(q3[:], [[1, R], [R, D]]), axis=X)

            # ---- warp select: warped = sum_h uP[p,h] * Z2[p,(h,i)] ----
            nc.vector.tensor_mul(selq[:].rearrange("p (h i) -> p h i", h=R),
                                 _fv(z2[:], [[NB, R], [1, NB]]),
                                 _fv(uP[:], [[1, R], [0, NB]]))
            nc.vector.reduce_sum(outb[:, 0:NB],
                                 _fv(selq[:], [[1, NB], [NB, R]]), axis=X)

            # ---- correlation: P1/P2 products, iDFT accumulate ----
            nc.scalar.copy(fsb[:], fps[:])
            nc.vector.tensor_mul(p1[:], fsb[:, 0:P], fsb[:, P:2 * P])
            nc.vector.tensor_mul(p2[:], fsb[:, 0:P], fsb[:, 2 * P:3 * P])
            nc.tensor.matmul(wfT[:], a72[:, C_ID1:C_ID1 + 37], p1[:],
                             start=True, stop=False)
            nc.tensor.matmul(wfT[:], a72[:, C_ID2:C_ID2 + 37], p2[:],
                             start=False, stop=True)

            # ---- transpose wf+sz to row layout ----
            nc.scalar.copy(asmT[:], wfT[:])
            nc.tensor.transpose(rowP[:], asmT[:],
                                a72[0:37, C_EYE:C_EYE + 37])

            # ---- logpost = ln(wf) - ln(sz) ----
            nc.scalar.activation(lnrow[:], rowP[:], act.Ln)
            nc.vector.tensor_scalar(outb[:, NB + 2:OC], lnrow[:, 0:NB],
                                    lnrow[:, 36:37], None, alu.subtract)

            # ---- population vector readout + normalize + clip ----
            nc.vector.tensor_mul(prdv[:].rearrange("p (k c) -> p k c", k=NB),
                                 _fv(rowP[:, 0:NB], [[1, NB], [0, 2]]),
                                 _fv(a112[:, C_POP:C_POP + 2 * NB],
                                     [[2, NB], [1, 2]]))
            nc.vector.reduce_sum(vecu[:], _fv(prdv[:], [[1, 2], [2, NB]]),
                                 axis=X)
            # vec_x += 1e-8 * sz (the reference adds eps before normalizing)
            nc.vector.scalar_tensor_tensor(vecu[:, 0:1], rowP[:, 36:37],
                                           1e-8, vecu[:, 0:1],
                                           alu.mult, alu.add)
            nc.vector.tensor_mul(sqx[:], vecu[:, 0:1], vecu[:, 0:1])
            nc.vector.scalar_tensor_tensor(n2[:], vecu[:, 1:2],
                                           vecu[:, 1:2], sqx[:],
                                           alu.mult, alu.add)
            nc.scalar.activation(lnn[:], n2[:], act.Ln)
            nc.scalar.activation(rn[:], lnn[:], act.Exp, scale=-0.5)
            nc.vector.tensor_scalar(vclip[:], vecu[:], rn[:, :1], 1.0,
                                    alu.mult, alu.min)
            nc.vector.tensor_scalar(outb[:, NB:NB + 2], vclip[:], -1.0,
                                    None, alu.max)

            # ---- store ----
            nc.sync.dma_start(d_out[:], outb[:])

    return nc


_NC_CACHE = {}


def _get_nc():
    nc = _NC_CACHE.get(0)
    if nc is None:
        nc = _emit(_Bacc(None, target_bir_lowering=False))
        nc.compile()
        _NC_CACHE[0] = nc
    return nc


_CONST_CACHE = {}


def _consts(transform_matrices, logprior_rotate_matrix, population_vector):
    """Host-side stationaries derived from the (constant) model tensors."""
    key = (transform_matrices.tobytes()[:256], logprior_rotate_matrix.tobytes()[:64])
    hit = _CONST_CACHE.get(key)
    if hit is not None:
        return hit
    f32 = np.float32
    T = np.asarray(transform_matrices, np.float64)
    M = np.asarray(logprior_rotate_matrix, np.float64)
    pop = np.asarray(population_vector, f32)

    U, Sv, Vt = np.linalg.svd(T.reshape(NA, -1), full_matrices=False)
    US = U[:, :R] * Sv[:R]
    angles = np.arange(-30, 31)
    V = np.vander(angles / 26.0, D, increasing=True)
    coef, *_ = np.linalg.lstsq(V, US, rcond=None)          # [D, R]
    Cb = Vt[:R].reshape(R, NB, NB)
    cbank = np.transpose(Cb, (2, 0, 1)).reshape(NB, R * NB)  # [j,(h,i)]

    n = np.arange(NB)
    ang = 2 * np.pi * np.outer(n, n) / NB
    Cm, Sm = np.cos(ang), np.sin(ang)
    dftA = np.concatenate([Cm, Sm], 1)                      # [i, 72]
    dftB = np.concatenate([Sm, Cm], 1)
    expf = np.exp(M[0, :])
    KK = Cm * expf[None, :] / NB                            # cos(2pi nk/36)*expf/36
    SS = Sm * expf[None, :] / NB
    id1 = np.concatenate([KK, KK], 0)                       # [72, 36]
    id2 = np.concatenate([SS, -SS], 0)
    id1 = np.concatenate([id1, id1.sum(1, keepdims=True)], 1)
    id2 = np.concatenate([id2, id2.sum(1, keepdims=True)], 1)

    a72 = np.zeros((72, W72), f32)
    a72[:, C_ID1:C_ID1 + 37] = id1
    a72[:, C_ID2:C_ID2 + 37] = id2
    a72[0:37, C_EYE:C_EYE + 37] = np.eye(37)

    dft2 = np.concatenate([dftA, dftB], 1).astype(f32)      # [36, 144]
    coefpop = np.concatenate(
        [coef.reshape(-1), pop.T.reshape(-1)]).astype(f32)  # [D*R + 72]
    out = (cbank.astype(ml_dtypes.bfloat16), dft2, a72, coefpop)
    _CONST_CACHE[key] = out
    return out


def _in_maps(loglikelihood1, loglikelihood2, inp, yaw,
             transform_matrices, logprior_rotate_matrix, template_log,
             population_vector):
    f32 = np.float32
    cbank, dft2, a72, coefpop = _consts(
        np.asarray(transform_matrices, f32),
        np.asarray(logprior_rotate_matrix, f32),
        np.asarray(population_vector, f32))
    ll1 = np.ascontiguousarray(loglikelihood1, f32)
    ll2 = np.ascontiguousarray(loglikelihood2, f32)
    inp = np.ascontiguousarray(inp, f32)
    yaw = np.ascontiguousarray(yaw, f32)

    coefpop_rep = np.tile(coefpop, (P, 1))                  # [112, D*R+72]
    maps = []
    for c in range(NCORES):
        bs = slice(BPC * c, BPC * (c + 1))
        x1t = ll1[bs].reshape(P, NB).T                      # [36, 112]
        x2t = ll2[bs].reshape(P, NB).T
        a36 = np.concatenate([x1t, x2t, dft2], axis=1)
        a112 = np.concatenate(
            [np.repeat(yaw[bs], S).reshape(P, 1), coefpop_rep], axis=1)
        b36 = np.concatenate(
            [cbank, inp[bs].reshape(P, NB).T.astype(ml_dtypes.bfloat16)],
            axis=1)
        maps.append({
            "a36": np.ascontiguousarray(a36),
            "a72": a72,
            "a112": np.ascontiguousarray(a112),
            "b36": np.ascontiguousarray(b36),
        })
    return maps


def run(trace=False, **inputs):
    """Run on 8 NeuronCores; returns (full_output, exec_time_ns_or_None)."""
    nc = _get_nc()
    maps = _in_maps(**inputs)
    res = run_bass_kernel_spmd(nc, maps, list(range(NCORES)), trace=trace)
    parts = [res.results[c]["out"].reshape(BPC, S, OC) for c in range(NCORES)]
    out = np.concatenate(parts, axis=0).astype(np.float32)
    return out, res.exec_time_ns


def kernel(**inputs):
    return run(trace=False, **inputs)[0]
